# revision 28
# baseline (speedup 1.0000x reference)
"""Single-head causal attention (B=8, T=2048, C=768, H=64) on 8 TRN2 NeuronCores.

Sharding: data-parallel over the batch dim — one batch element per core.

Per-core algorithm (bf16 matmul operands, fp32 PSUM accumulation):
  - inputs fed transposed + pre-cast to bf16 from the host: xT [C, T].
  - x DMA'd col-major as 6 pieces issued concurrently from BOTH HWDGE
    engines (sync + scalar) — a single InstDMACopy only sustains ~135 GB/s,
    so ring-level concurrency is what reaches the ~270+ GB/s aggregate, and
    early columns lead each ring so QKV group 0 starts first.
  - ScalarE runs the exp activations plus the two critical qkT casts
    (pre-attention, its queue is otherwise empty then); gpsimd-SWDGE
    carries the k^T shifts and output evacuations (ring-independent of the
    bulk x). VectorE does the other PSUM casts, masks and evac copies.
  - exp table preloaded at t=0; 10 warmup matmuls bridge the x wait and
    keep the PE HAM activity monitor busy (2.4 GHz by the time QKV runs).
  - qkT [128, T]: rows 0:64 = q^T, 64:128 = k^T (fused [Wq | Wk] weights);
    k^T shifted to a base-0 tile via SBUF->SBUF DMA per 512-col group.
  - vT80 [80, T]: rows 0:64 = v^T (row 64 unused here). Natural [v_j | 1]
    tiles v80 [128, 16, 80] via PE transposes (ones column memset), emitted
    in small slices between attention pairs.
  - attention in S^T layout (keys j on partitions, queries i on free), with
    QKV groups 2,3 and all v work injected into the pair stream in slices
    sized to the per-pair ScalarE slack, so the exp stream starts right
    after qk group 1 and stays dense.
  - AV: out^T [65, half] += [v_j | 1].T @ expS^T_j per 512-col half (own
    PSUM bank); row 64 accumulates softmax denominators; halves evacuated
    (PSUM -> SBUF bf16) as soon as their last j-chunk lands, then DMA'd out.
  - output is oT [65, T] bf16 (unnormalized + denominators); the host does
    out = (oT[:64] / oT[64:65]).T in fp32 — no device-side finalize.

No max-subtraction in softmax: scores * C**-0.5 are bounded (|s| < ~3), exp is
safe in fp32, and the result is mathematically identical to jax.nn.softmax.
"""

import ml_dtypes
import numpy as np

import concourse.bass as bass
import concourse.tile as tile
from concourse import bacc, mybir
from concourse.bass import ds, ts
from concourse.masks import make_identity, make_upper_triangular

B, T, C, H = 8, 2048, 768, 64
P = 128
NCH = C // P          # 6 contraction chunks for QKV
GW = 1024             # attention output column-group width
NG = T // GW          # 2 groups
NT = T // P           # 16 t-chunks
JPG = GW // P         # 8 j-chunks per group
VP = 80               # vT partition rows (64 v + pad to 16x for tile pools)
SCALE = float(C) ** -0.5
N_WARMUP = 12

F32 = mybir.dt.float32
BF16 = mybir.dt.bfloat16
EXP = mybir.ActivationFunctionType.Exp


def _emit(tc: tile.TileContext, ctx, xT, wqk, wv, oT):
    nc = tc.nc

    consts = ctx.enter_context(tc.tile_pool(name="consts", bufs=1))
    xpool = ctx.enter_context(tc.tile_pool(name="x", bufs=1))
    qpool = ctx.enter_context(tc.tile_pool(name="qkv", bufs=1))

    # ---- t=0: input DMAs (both HWDGE engines), exp-table preload ----
    xT_sb = xpool.tile([P, NCH, T], BF16)
    xTr = xT.rearrange("(o p) t -> p o t", p=P)

    def xpiece(eng, c0, c1, t0, t1):
        eng.dma_start(xT_sb[:, c0:c1, t0:t1], xTr[:, c0:c1, t0:t1])

    # preload the exp table set first (scalar engine, overlaps everything)
    dummy = consts.tile([P, 1], F32)
    nc.vector.memset(dummy[:], 0.0)
    nc.scalar.activation(dummy[:], dummy[:], EXP)
    # weights lead their rings: they gate the very first QKV LDWEIGHTS
    w_qk = consts.tile([P, NCH, P], BF16)
    nc.sync.dma_start(w_qk[:], wqk.rearrange("(o p) m -> p o m", p=P))
    w_v = consts.tile([P, NCH, H], BF16)
    nc.scalar.dma_start(w_v[:], wv.rearrange("(o p) m -> p o m", p=P))
    # col-major pieces, alternating rings: ring-order pipelining means the
    # early pieces complete first (~2 in flight per ring, ~135 GB/s each)
    xpiece(nc.sync, 0, 3, 0, 512)
    xpiece(nc.scalar, 3, 6, 0, 512)
    xpiece(nc.sync, 0, 3, 512, 1024)
    xpiece(nc.scalar, 3, 6, 512, 1024)
    xpiece(nc.sync, 0, 6, 1024, 1536)
    xpiece(nc.scalar, 0, 6, 1536, 2048)

    # warmup tile for dummy matmuls (gpsimd memset: that engine clears its
    # preamble earliest, so the warmup can start ~1us sooner)
    dum = qpool.tile([P, 512], BF16)
    nc.gpsimd.memset(dum[:], 0.0)

    ident = consts.tile([H, H], BF16)
    make_identity(nc, ident[:])
    tri = consts.tile([P, P], BF16)
    make_upper_triangular(nc, tri[:], val=1.0, diag=True)

    qkT = qpool.tile([P, T], BF16)
    kT = qpool.tile([H, T], BF16)
    vT80 = qpool.tile([VP, T], BF16)
    v80 = qpool.tile([P, NT, VP], BF16)
    # ones column for the AV denominator rows
    nc.vector.memset(v80[:, :, H : H + 1], 1.0)

    qk_ps = {}
    v_ps = {}

    def qk_mm(g, cs, pool, tag):
        if g not in qk_ps:
            qk_ps[g] = pool.tile([P, 512], F32, tag=tag, name=f"qk_{g}")
        ps = qk_ps[g]
        for c in cs:
            nc.tensor.matmul(
                ps[:],
                w_qk[:, c, :],
                xT_sb[:, c, ts(g, 512)],
                start=(c == 0),
                stop=(c == NCH - 1),
            )

    def qk_fin(g, shift_eng, cast_eng=None):
        if cast_eng is nc.scalar:
            nc.scalar.copy(qkT[:, ts(g, 512)], qk_ps[g][:])
        else:
            nc.vector.tensor_copy(qkT[:, ts(g, 512)], qk_ps[g][:])
        # k^T lives at partitions 64:128; shift to base partition 0
        shift_eng.dma_start(kT[:, ts(g, 512)], qkT[H:P, ts(g, 512)])

    def v_mm(g, cs, pool, tag):
        if g not in v_ps:
            v_ps[g] = pool.tile([P, 512], F32, tag=tag, name=f"v_{g}")
        ps = v_ps[g][0:H, :]
        for c in cs:
            nc.tensor.matmul(
                ps,
                w_v[:, c, :],
                xT_sb[:, c, ts(g, 512)],
                start=(c == 0),
                stop=(c == NCH - 1),
            )

    def v_fin(g):
        nc.vector.tensor_copy(vT80[0:H, ts(g, 512)], v_ps[g][0:H, :])

    # pool A: warmup + qk groups 0,1 (gates scores); closed before attention
    with tc.tile_pool(name="papsum", bufs=2, space="PSUM") as pa:
        for w in range(N_WARMUP):
            dps = pa.tile([P, 512], F32, tag="qk", name=f"warm_{w}")
            nc.tensor.matmul(dps[:], dum[:, 0:P], dum[:], start=True, stop=True)
        qk_mm(0, range(3), pa, "qk")
        qk_mm(0, range(3, 6), pa, "qk")
        qk_fin(0, nc.gpsimd, nc.scalar)
        qk_mm(1, range(3), pa, "qk")
        qk_mm(1, range(3, 6), pa, "qk")
        qk_fin(1, nc.gpsimd, nc.scalar)

    # pool B: v matmuls + late qk (one shared bank) + PE-transpose staging
    pq = ctx.enter_context(tc.tile_pool(name="pbpsum", bufs=1, space="PSUM"))

    def pe_transpose(t):
        pt = pq.tile([P, H], BF16, tag="vt", name=f"vt_{t}")
        nc.tensor.transpose(pt[:], vT80[0:H, ts(t, P)], ident[:])
        nc.vector.tensor_copy(v80[:, t, 0:H], pt[:])

    # ---- attention (remaining QKV injected in small slices) ----
    sp = ctx.enter_context(tc.tile_pool(name="spsum", bufs=2, space="PSUM"))
    op = ctx.enter_context(tc.tile_pool(name="opsum", bufs=2, space="PSUM"))
    pb = ctx.enter_context(tc.tile_pool(name="probs", bufs=6))
    fin = ctx.enter_context(tc.tile_pool(name="fin", bufs=3))

    inject = {
        # all of v0/v1 + their transposes go before the pair stream: the
        # scheduler slots them into the PE idle window while the first k^T
        # shift is in flight, emptying the exp window of that work
        -1: lambda: (v_mm(0, range(6), pq, "v"), v_fin(0),
                     pe_transpose(0), pe_transpose(1), pe_transpose(2),
                     pe_transpose(3),
                     v_mm(1, range(6), pq, "v"), v_fin(1),
                     pe_transpose(4), pe_transpose(5), pe_transpose(6),
                     pe_transpose(7)),
        # late qk in thin 2-chunk slices against the per-pair ScalarE slack
        0: lambda: qk_mm(2, range(2), pq, "v"),
        1: lambda: qk_mm(2, range(2, 4), pq, "v"),
        2: lambda: (qk_mm(2, range(4, 6), pq, "v"), qk_fin(2, nc.gpsimd)),
        3: lambda: qk_mm(3, range(2), pq, "v"),
        4: lambda: qk_mm(3, range(2, 4), pq, "v"),
        5: lambda: (qk_mm(3, range(4, 6), pq, "v"), qk_fin(3, nc.gpsimd)),
        8: lambda: v_mm(2, range(3), pq, "v"),
        9: lambda: (v_mm(2, range(3, 6), pq, "v"), v_fin(2)),
        10: lambda: (pe_transpose(8), pe_transpose(9)),
        11: lambda: (pe_transpose(10), pe_transpose(11)),
        12: lambda: v_mm(3, range(3), pq, "v"),
        13: lambda: (v_mm(3, range(3, 6), pq, "v"), v_fin(3)),
        14: lambda: (pe_transpose(12), pe_transpose(13)),
        15: lambda: (pe_transpose(14), pe_transpose(15)),
    }

    def emit_probs(g, jj):
        istart = max(g * GW, jj * P)
        n = (g + 1) * GW - istart
        sps = sp.tile([P, GW], F32, tag="s")
        for h in range(0, n, 512):
            nh = min(512, n - h)
            nc.tensor.matmul(
                sps[:, h : h + nh],
                kT[:, ts(jj, P)],
                qkT[0:H, ds(istart + h, nh)],
                start=True,
                stop=True,
            )
        prb = pb.tile([P, GW], BF16, tag="p")
        nc.scalar.activation(prb[:, :n], sps[:, :n], EXP, scale=SCALE)
        if jj >= JPG * g:
            # leading 128 cols are the diagonal block: upper-tri (j<=i) mask
            nc.vector.tensor_mul(out=prb[:, :P], in0=prb[:, :P], in1=tri[:])
        return [(prb, 0)]

    def emit_probs2(g, jj):
        # merged unit: pairs (g,jj) and (g,jj+1) share one staging tile and
        # ONE activate. Only legal when pair 1 exactly fills PSUM bank A
        # (n1 == 512): each start=True matmul clears has_written for its
        # whole bank, so the two pairs must land in different banks.
        n2 = (g + 1) * GW - (jj + 1) * P
        sps = sp.tile([P, GW], F32, tag="s")
        for k, off in ((jj, 0), (jj + 1, 512)):
            nw = (g + 1) * GW - k * P if off else 512
            nc.tensor.matmul(
                sps[:, off : off + nw],
                kT[:, ts(k, P)],
                qkT[0:H, ds(k * P, nw)],
                start=True,
                stop=True,
            )
        prb = pb.tile([P, GW], BF16, tag="p")
        nc.scalar.activation(prb[:, : 512 + n2], sps[:, : 512 + n2], EXP,
                             scale=SCALE)
        for off in (0, 512):
            nc.vector.tensor_mul(
                out=prb[:, off : off + P], in0=prb[:, off : off + P],
                in1=tri[:],
            )
        return [(prb, 0), (prb, 512)]

    def emit_evac(g, hh, oph, last):
        osb = fin.tile([H + 1, 512], BF16, tag="osb", name=f"osb_{g}_{hh}")
        nc.vector.tensor_copy(osb[:], oph[:])
        eng = nc.sync if last else nc.gpsimd
        eng.dma_start(oT[:, ds(g * GW + hh * 512, 512)], osb[:])

    pairs = [(g, jj) for g in range(NG) for jj in range(JPG * g + JPG)]
    # the two diagonal pairs whose first member exactly fills a PSUM bank
    # are emitted merged (one activate); second members are skipped
    merged = {(0, 4), (1, 12)}
    emits = []
    for g, jj in pairs:
        if (g, jj) in merged:
            emits.append(lambda g=g, jj=jj: emit_probs2(g, jj))
        elif (g, jj - 1) not in merged:
            emits.append(lambda g=g, jj=jj: emit_probs(g, jj))
    ops_by_gh = {}
    LOOKAHEAD = 3
    prb_queue = []
    ei = 0
    while len(prb_queue) < LOOKAHEAD and ei < len(emits):
        prb_queue.extend(emits[ei]())
        ei += 1
    inject[-1]()
    for idx, (g, jj) in enumerate(pairs):
        prb, poff = prb_queue.pop(0)

        if jj == 0:
            for hh in range(2):
                ops_by_gh[(g, hh)] = op.tile(
                    [H + 1, 512], F32, tag="o", name=f"ops_{g}_{hh}"
                )
        istart = max(g * GW, jj * P)
        n = (g + 1) * GW - istart
        ioff = istart - g * GW
        seg = ioff
        while seg < ioff + n:
            seg_end = min(ioff + n, (seg // 512 + 1) * 512)
            half = seg // 512
            # last j-chunk writing this 512-wide half of the group
            jj_last = min(JPG * g + JPG - 1, JPG * g + 4 * (half + 1) - 1)
            oph = ops_by_gh[(g, half)]
            nc.tensor.matmul(
                oph[:, seg - half * 512 : seg_end - half * 512],
                v80[:, jj, 0 : H + 1],
                prb[:, poff + seg - ioff : poff + seg_end - ioff],
                start=(jj == 0),
                stop=(jj == jj_last),
            )
            if jj == jj_last:
                emit_evac(g, half, oph, last=(idx == len(pairs) - 1))
            seg = seg_end

        if idx in inject:
            inject[idx]()
        while len(prb_queue) < LOOKAHEAD and ei < len(emits):
            prb_queue.extend(emits[ei]())
            ei += 1


def build():
    from contextlib import ExitStack

    nc = bacc.Bacc("TRN2", target_bir_lowering=False, debug=False, num_devices=B)
    xT = nc.dram_tensor("xT", [C, T], BF16, kind="ExternalInput").ap()
    wqk = nc.dram_tensor("wqk", [C, P], BF16, kind="ExternalInput").ap()
    wv = nc.dram_tensor("wv", [C, H], BF16, kind="ExternalInput").ap()
    oT = nc.dram_tensor("oT", [H + 1, T], BF16, kind="ExternalOutput").ap()
    with tile.TileContext(nc) as tc, ExitStack() as ctx:
        _emit(tc, ctx, xT, wqk, wv, oT)
    nc.compile()
    return nc


_NC = None


def _get_nc():
    global _NC
    if _NC is None:
        _NC = build()
    return _NC


def make_in_maps(x, Wk, Wq, Wv):
    bf = ml_dtypes.bfloat16
    wqk = np.ascontiguousarray(np.concatenate([Wq, Wk], axis=1)).astype(bf)
    wv = np.ascontiguousarray(np.asarray(Wv)).astype(bf)
    return [
        {
            "xT": np.ascontiguousarray(np.asarray(x[b]).T).astype(bf),
            "wqk": wqk,
            "wv": wv,
        }
        for b in range(B)
    ]


def finalize_host(oT):
    """oT [65, T] bf16 -> normalized [T, H] fp32 output."""
    oT = np.asarray(oT, dtype=np.float32)
    return np.ascontiguousarray((oT[:H] / oT[H : H + 1]).T, dtype=np.float32)


def kernel(x, Wk, Wq, Wv):
    from concourse.bass_utils import run_bass_kernel_spmd

    nc = _get_nc()
    in_maps = make_in_maps(x, Wk, Wq, Wv)
    r = run_bass_kernel_spmd(nc, in_maps, core_ids=list(range(B)))
    out = np.stack([finalize_host(r.results[b]["oT"]) for b in range(B)])
    return np.ascontiguousarray(out, dtype=np.float32)
